# revision 17
# baseline (speedup 1.0000x reference)
"""AttnNet kernel for Trainium2: attn = softmax(einsum("bsh,bh->bs", facts, questions))[:, None, :].

Full shapes: questions [64, 4096] f32, facts [64, 512, 4096] f32 -> out [64, 1, 512] f32.
Data-parallel over batch: 8 batches per NeuronCore x 8 cores, no collectives.

Per-core dataflow (B_LOC=8, S=512, H=4096):
  - facts streamed as 32 contiguous [128(s), 4096(h)] tiles (2 MiB each) on the
    sync HWDGE ring only.
  - q[0] replicated to 128 partitions via a DMA broadcast read on the scalar
    ring (dodges the ~15 us gpsimd cold-start); q[1..7] via gpsimd
    partition_broadcast.
  - Fused DVE tensor_tensor_reduce per tile: (facts*q) row-sum -> E[:, col],
    with the full-size product dumped to PSUM (keeps the SBUF ports free for
    the DMA stream; the kernel is SBUF-bandwidth bound otherwise).
  - Epilogue: DVE 32x32 block transposes E [128,32] -> [32,128] (SBUF only, no
    PE/PSUM), regroup to [8, 512] via SBUF->SBUF DMA, then softmax: -max (DVE),
    fused exp+sum (ACT), reciprocal + scale (DVE), DMA out.
"""

import numpy as np

B, S, H = 64, 512, 4096
N_CORES = 8
B_LOC = B // N_CORES  # 8
P = 128
SC = S // P  # 4 s-chunks per batch

_CACHE = {}


def _build_bass():
    import concourse.bacc as bacc
    import concourse.mybir as mybir
    import concourse.tile as tile

    f32 = mybir.dt.float32

    nc = bacc.Bacc("TRN2", target_bir_lowering=False, debug=False)
    facts = nc.dram_tensor("facts", [B_LOC, S, H], f32, kind="ExternalInput").ap()
    questions = nc.dram_tensor("questions", [B_LOC, H], f32, kind="ExternalInput").ap()
    attn = nc.dram_tensor("attn", [B_LOC, S], f32, kind="ExternalOutput").ap()

    with tile.TileContext(nc) as tc:
        with (
            tc.tile_pool(name="consts", bufs=1) as consts,
            tc.tile_pool(name="fpool", bufs=7) as fpool,
            tc.tile_pool(name="qrow", bufs=2) as qrow,
            tc.tile_pool(name="qsb", bufs=2) as qsb,
            tc.tile_pool(name="pq", bufs=1, space="PSUM") as pqpool,
        ):
            # energies, column b*SC+c holds energies[b, c*128:(c+1)*128] on partitions
            E = consts.tile([P, B_LOC * SC], f32)
            # ACT's copy-out target lives in PSUM: its 2 MiB/chunk write stays
            # off the SBUF ports, which the DMA stream and DVE need (the kernel
            # is SBUF-bandwidth bound otherwise). Only ACT touches it, and ACT
            # executes in order, so one buffer is safe.
            dump = pqpool.tile([P, H], f32)

            for b in range(B_LOC):
                q_b = qsb.tile([P, H], f32)
                if b == 0:
                    # replicate q[0] across partitions straight from HBM on the
                    # scalar ring: ready long before the gpsimd Q7 cores warm up
                    nc.scalar.dma_start(
                        out=q_b[:], in_=questions[0:1, :].partition_broadcast(P)
                    )
                else:
                    q_row = qrow.tile([1, H], f32)
                    nc.scalar.dma_start(out=q_row[:], in_=questions[b : b + 1, :])
                    nc.gpsimd.partition_broadcast(q_b[:], q_row[:])

                for c in range(SC):
                    ftile = fpool.tile([P, H], f32)
                    nc.sync.dma_start(
                        out=ftile[:], in_=facts[b, c * P : (c + 1) * P, :]
                    )
                    col = b * SC + c
                    # in-place multiply (fastest DVE mode: 2 SBUF regions)
                    nc.vector.tensor_mul(out=ftile[:], in0=ftile[:], in1=q_b[:])
                    # ACT fused copy+accumulate: accum_out = row sum; the
                    # full-size copy-out is dumped to PSUM
                    nc.scalar.activation(
                        dump[:],
                        ftile[:],
                        mybir.ActivationFunctionType.Copy,
                        accum_out=E[:, col : col + 1],
                    )

            # --- softmax epilogue (no PE) ---
            # DVE 32x32 block transposes: E [128, 32] -> e_t [32, 128]
            e_t = consts.tile([B_LOC * SC, P], f32)
            for r in range(4):
                nc.vector.transpose(
                    e_t[:, 32 * r : 32 * (r + 1)], E[32 * r : 32 * (r + 1), :]
                )
            # regroup [32, 128] (p = b*4+c) -> [8, 512]
            e_rows = consts.tile([B_LOC, S], f32)
            nc.sync.dma_start(
                out=e_rows[:].rearrange("b (c i) -> b c i", i=P), in_=e_t[:]
            )

            neg_max = consts.tile([B_LOC, 1], f32)
            nc.vector.reduce_max(
                neg_max[:], e_rows[:], axis=mybir.AxisListType.X, negate=True
            )

            p_exp = consts.tile([B_LOC, S], f32)
            den = consts.tile([B_LOC, 1], f32)
            nc.scalar.activation(
                p_exp[:],
                e_rows[:],
                mybir.ActivationFunctionType.Exp,
                bias=neg_max[:],
                scale=1.0,
                accum_out=den[:],
            )

            recip = consts.tile([B_LOC, 1], f32)
            nc.vector.reciprocal(recip[:], den[:])

            a_t = consts.tile([B_LOC, S], f32)
            nc.vector.tensor_scalar_mul(a_t[:], p_exp[:], recip[:])

            nc.sync.dma_start(out=attn, in_=a_t[:])

    nc.compile()
    return nc


def _get_nc():
    if "nc" not in _CACHE:
        _CACHE["nc"] = _build_bass()
    return _CACHE["nc"]


def _shard_inputs(questions, facts):
    questions = np.ascontiguousarray(np.asarray(questions), dtype=np.float32)
    facts = np.ascontiguousarray(np.asarray(facts), dtype=np.float32)
    in_maps = []
    for i in range(N_CORES):
        sl = slice(i * B_LOC, (i + 1) * B_LOC)
        in_maps.append(
            {
                "facts": np.ascontiguousarray(facts[sl]),
                "questions": np.ascontiguousarray(questions[sl]),
            }
        )
    return in_maps


def _run(questions, facts, **run_kwargs):
    from concourse.bass_utils import run_bass_kernel_spmd

    nc = _get_nc()
    in_maps = _shard_inputs(questions, facts)
    res = run_bass_kernel_spmd(nc, in_maps, core_ids=list(range(N_CORES)), **run_kwargs)
    out = np.stack([np.asarray(res.results[i]["attn"]) for i in range(N_CORES)])
    return out.reshape(B, S)[:, None, :].astype(np.float32), res


def kernel(questions, facts):
    out, _ = _run(questions, facts)
    return out


# revision 18
# speedup vs baseline: 1.0161x; 1.0161x over previous
"""AttnNet kernel for Trainium2: attn = softmax(einsum("bsh,bh->bs", facts, questions))[:, None, :].

Full shapes: questions [64, 4096] f32, facts [64, 512, 4096] f32 -> out [64, 1, 512] f32.
Data-parallel over batch: 8 batches per NeuronCore x 8 cores, no collectives.

Per-core dataflow (B_LOC=8, S=512, H=4096):
  - facts streamed as 32 contiguous [128(s), 4096(h)] tiles (2 MiB each) on the
    sync HWDGE ring only.
  - q[0] replicated to 128 partitions via a DMA broadcast read on the scalar
    ring (dodges the ~15 us gpsimd cold-start); q[1..7] via gpsimd
    partition_broadcast.
  - Fused DVE tensor_tensor_reduce per tile: (facts*q) row-sum -> E[:, col],
    with the full-size product dumped to PSUM (keeps the SBUF ports free for
    the DMA stream; the kernel is SBUF-bandwidth bound otherwise).
  - Epilogue: DVE 32x32 block transposes E [128,32] -> [32,128] (SBUF only, no
    PE/PSUM), regroup to [8, 512] via SBUF->SBUF DMA, then softmax: -max (DVE),
    fused exp+sum (ACT), reciprocal + scale (DVE), DMA out.
"""

import numpy as np

B, S, H = 64, 512, 4096
N_CORES = 8
B_LOC = B // N_CORES  # 8
P = 128
SC = S // P  # 4 s-chunks per batch

_CACHE = {}


def _build_bass():
    import concourse.bacc as bacc
    import concourse.mybir as mybir
    import concourse.tile as tile

    f32 = mybir.dt.float32

    nc = bacc.Bacc("TRN2", target_bir_lowering=False, debug=False)
    facts = nc.dram_tensor("facts", [B_LOC, S, H], f32, kind="ExternalInput").ap()
    questions = nc.dram_tensor("questions", [B_LOC, H], f32, kind="ExternalInput").ap()
    attn = nc.dram_tensor("attn", [B_LOC, S], f32, kind="ExternalOutput").ap()

    with tile.TileContext(nc) as tc:
        with (
            tc.tile_pool(name="consts", bufs=1) as consts,
            tc.tile_pool(name="fpool", bufs=7) as fpool,
            tc.tile_pool(name="qrow", bufs=2) as qrow,
            tc.tile_pool(name="qsb", bufs=2) as qsb,
            tc.tile_pool(name="pq", bufs=1, space="PSUM") as pqpool,
        ):
            # energies, column b*SC+c holds energies[b, c*128:(c+1)*128] on partitions
            E = consts.tile([P, B_LOC * SC], f32)
            # the fused op's copy-out target lives in PSUM: its 2 MiB/chunk
            # write stays off the SBUF ports, which the DMA stream needs (the
            # kernel is SBUF-bandwidth bound otherwise). Only DVE touches it,
            # in order, so one buffer is safe.
            dump = pqpool.tile([P, H], f32)

            for b in range(B_LOC):
                q_b = qsb.tile([P, H], f32)
                if b == 0:
                    # replicate q[0] across partitions straight from HBM on the
                    # scalar ring: ready long before the gpsimd Q7 cores warm up
                    nc.scalar.dma_start(
                        out=q_b[:], in_=questions[0:1, :].partition_broadcast(P)
                    )
                else:
                    q_row = qrow.tile([1, H], f32)
                    nc.scalar.dma_start(out=q_row[:], in_=questions[b : b + 1, :])
                    nc.gpsimd.partition_broadcast(q_b[:], q_row[:])

                for c in range(SC):
                    ftile = fpool.tile([P, H], f32)
                    nc.sync.dma_start(
                        out=ftile[:], in_=facts[b, c * P : (c + 1) * P, :]
                    )
                    col = b * SC + c
                    # fused multiply + row-sum in one DVE pass (custom DVE op;
                    # the native TensorTensorReduce opcode crashes on HW).
                    # ACT does no per-tile work at all.
                    nc.vector.affine_mul_reduce(
                        out=dump[:],
                        accum_out=E[:, col : col + 1],
                        in0=ftile[:],
                        in1=q_b[:],
                        scale=1.0,
                        bias=0.0,
                    )

            # --- softmax epilogue (no PE) ---
            # DVE 32x32 block transposes: E [128, 32] -> e_t [32, 128]
            e_t = consts.tile([B_LOC * SC, P], f32)
            for r in range(4):
                nc.vector.transpose(
                    e_t[:, 32 * r : 32 * (r + 1)], E[32 * r : 32 * (r + 1), :]
                )
            # regroup [32, 128] (p = b*4+c) -> [8, 512]
            e_rows = consts.tile([B_LOC, S], f32)
            nc.sync.dma_start(
                out=e_rows[:].rearrange("b (c i) -> b c i", i=P), in_=e_t[:]
            )

            neg_max = consts.tile([B_LOC, 1], f32)
            nc.vector.reduce_max(
                neg_max[:], e_rows[:], axis=mybir.AxisListType.X, negate=True
            )

            p_exp = consts.tile([B_LOC, S], f32)
            den = consts.tile([B_LOC, 1], f32)
            nc.scalar.activation(
                p_exp[:],
                e_rows[:],
                mybir.ActivationFunctionType.Exp,
                bias=neg_max[:],
                scale=1.0,
                accum_out=den[:],
            )

            recip = consts.tile([B_LOC, 1], f32)
            nc.vector.reciprocal(recip[:], den[:])

            a_t = consts.tile([B_LOC, S], f32)
            nc.vector.tensor_scalar_mul(a_t[:], p_exp[:], recip[:])

            nc.sync.dma_start(out=attn, in_=a_t[:])

    nc.compile()
    return nc


def _get_nc():
    if "nc" not in _CACHE:
        _CACHE["nc"] = _build_bass()
    return _CACHE["nc"]


def _shard_inputs(questions, facts):
    questions = np.ascontiguousarray(np.asarray(questions), dtype=np.float32)
    facts = np.ascontiguousarray(np.asarray(facts), dtype=np.float32)
    in_maps = []
    for i in range(N_CORES):
        sl = slice(i * B_LOC, (i + 1) * B_LOC)
        in_maps.append(
            {
                "facts": np.ascontiguousarray(facts[sl]),
                "questions": np.ascontiguousarray(questions[sl]),
            }
        )
    return in_maps


def _run(questions, facts, **run_kwargs):
    from concourse.bass_utils import run_bass_kernel_spmd

    nc = _get_nc()
    in_maps = _shard_inputs(questions, facts)
    res = run_bass_kernel_spmd(nc, in_maps, core_ids=list(range(N_CORES)), **run_kwargs)
    out = np.stack([np.asarray(res.results[i]["attn"]) for i in range(N_CORES)])
    return out.reshape(B, S)[:, None, :].astype(np.float32), res


def kernel(questions, facts):
    out, _ = _run(questions, facts)
    return out
